# revision 1
# baseline (speedup 1.0000x reference)
"""Trainium2 Bass kernel for nn_CanadarmJacob (space-arm Jacobian, bm=1 path).

Contract: kernel(**inputs) takes FULL inputs (com_list (512,256,3,7) f32,
link_pose_list (512,256,4,4,9) f32, bm scalar) and returns the FULL output
(512,256,6,7) f32. Internally shards samples across 8 NeuronCores (pure data
parallel), runs a Bass/Tile kernel per core, and gathers.

Math (reformulated from the reference):
  pos   = pose[:3, 3, :7]
  rot   = pose[:3, AXIS[a], a] with AXIS=[2,0,2,2,2,0,2], rot[:,4] *= -1
  del   = com - pos
  jacob = rot x del                          (per-act cross product)
  w_k   = sum_{a>=k} M_a del_a               (suffix mass-weighted cumsum)
  Hphi  = D_suf ⊙ rot + w x jacob            (D_suf = suffix inertia diags)
  S_cc' = sum_a M_a del_c del_c'             (6 unique entries)
  c     = (sum_a M_a com_a)/TM - BASE
  H_s   = TM(c c^T - |c|^2 I) + CD + (Sxx+Syy+Szz) I - S
  jsm   = SM_k jacob_k                       (SM = suffix masses)
  Hth   = Hphi - c x jsm
  A     = -inv(H_s)   (symmetric 3x3, via adjugate and ACT reciprocal)
  bot   = A @ Hth
  top   = -(1/TM) jsm + c x bot
  out   = concat(top, bot) rows
"""
import sys
import functools

if "/opt/trn_rl_repo" not in sys.path:
    sys.path.insert(0, "/opt/trn_rl_repo")

import numpy as np

# ---------------------------------------------------------------- constants
N_CORES = 8
P = 128          # SBUF partitions
J = 128          # samples per partition per core
S_CORE = P * J   # 16384 samples per core
N_ACT = 7

MASS = np.array([105.98, 105.98, 314.98, 279.2, 105.98, 105.98, 243.66], np.float64)
TM = float(MASS.sum() + 100000.0 + 243.66)
DIAGS = np.array([[12.19, 12.19, 3.061], [12.19, 12.19, 3.061], [15.41, 2094.71, 2103.19],
                  [9.522, 1966.28, 1966.28], [8.305, 3.061, 8.0386], [12.13, 12.13, 3.061],
                  [9.336, 44.41, 44.41]], np.float64)
D_SUF = np.cumsum(DIAGS[::-1], axis=0)[::-1]          # (7,3) suffix inertia diag
SM = np.cumsum(MASS[::-1])[::-1]                      # (7,) suffix mass
CD = DIAGS.sum(axis=0)                                # (3,)
_TF0 = np.array([[1, 0, 0, 0], [0, -1, 0, 0], [0, 0, 1.3, 6], [0, 0, 0, 1]], np.float64)
_COM0 = np.array([[1, 0, 0, 0], [0, 1, 0, 0], [0, 0, 1, 0.5], [0, 0, 0, 1]], np.float64)
BASE = (_TF0 @ _COM0)[:3, 3] * 243.66 / (100000.0 + 243.66)   # [0, 0, ~0.0162]

# consts row layout (replicated to all 128 partitions host-side)
#   [0:7] M, [7:14] SM, [14:35] D (c-major: D[c][a]), [35:38] CD
CONSTS = np.concatenate([MASS, SM, D_SUF.T.reshape(-1), CD]).astype(np.float32)
NCONST = CONSTS.shape[0]

# smalls tile row indices (each row is (128, J) f32)
SS_R, CSQ_R = 0, 1
CC = 2            # rows 2..4 diag (xx,yy,zz), 5..7 off (xy,xz,yz)
HS = 8            # rows 8..13: [xx,yy,zz,xy,xz,yz]
ADJ = 14          # rows 14..19: [a11,a22,a33,a12,a13,a23]
M1_R, M2_R = 20, 22
T0_R, T1_R, T2_R = 24, 25, 26
DET_R, RDN_R = 27, 28
NSMALL = 29


def _emit(nc, tc, ctx, dram):
    import concourse.bass as bass
    from concourse import mybir

    f32 = mybir.dt.float32
    OP = mybir.AluOpType
    AX = mybir.AxisListType
    V = nc.vector
    G = nc.gpsimd

    NCH = 4                       # pose DMA / early-stage pipeline chunks
    CJ = J // NCH

    pool = ctx.enter_context(tc.tile_pool(name="main", bufs=1))
    ppool = ctx.enter_context(tc.tile_pool(name="pp", bufs=NCH))

    consts = pool.tile([P, NCONST], f32)
    # pose chunks + outb rotate through the same NCH slots (sized by outb)
    poses = [ppool.tile([P, CJ, 144], f32, tag="px", name=f"pose{h}")
             for h in range(NCH)]
    com = pool.tile([P, J, 21], f32, tag="com")
    delb = pool.tile([P, 3, J, N_ACT], f32, tag="dj")
    mw = pool.tile([P, 3, J, N_ACT], f32)     # mdel, suffix-summed in place -> w
    jac = pool.tile([P, 3, J, N_ACT], f32)
    hphi = pool.tile([P, 3, J, N_ACT], f32)   # Hphi -> Htheta in place
    prod = pool.tile([P, 9, J, N_ACT], f32)   # 6 S-products + 3 mcom; later scratch
    cv = pool.tile([P, 3, J], f32)
    smalls = pool.tile([P, NSMALL, J], f32)
    outb = ppool.tile([P, J, 42], f32, tag="px")    # rotates onto pose slots
    red = pool.tile([P, 9, J], f32, tag="com")      # reuses com slot

    # All input DMAs on the sync HWDGE ring (FIFO). consts+com are small and
    # gate the first del chunk, so they go first; pose chunks follow and the
    # chunk-h compute pipelines behind them.
    nc.sync.dma_start(out=consts[:], in_=dram["consts"][:])
    nc.sync.dma_start(out=com[:], in_=dram["com"][:])
    for h in range(NCH):
        nc.sync.dma_start(out=poses[h][:],
                          in_=dram["pose"][:, h * CJ:(h + 1) * CJ, :])

    # handy views
    comR = com[:].rearrange("p j (c a) -> p c j a", c=3)          # (P,3,J,7)

    def bc(ap, shape):
        return ap.broadcast_to(shape)

    Mb = bc(consts[:, 0:7].unsqueeze(1).unsqueeze(2), (P, 3, CJ, N_ACT))
    SMb = bc(consts[:, 7:14].unsqueeze(1).unsqueeze(2), (P, 3, J, N_ACT))
    Db = bc(consts[:, 14:35].rearrange("p (c a) -> p c a", c=3).unsqueeze(2),
            (P, 3, J, N_ACT))
    CDb = bc(consts[:, 35:38].unsqueeze(2), (P, 3, J))

    # early stages per pose chunk, pipelined behind the chunk DMAs
    rots = []
    for h in range(NCH):
        js = slice(h * CJ, (h + 1) * CJ)
        poseR = poses[h][:].rearrange("p j (r k) -> p r j k", r=4)[:, 0:3]
        # rot = axis-col-2 block patched in place: acts 1,5 from axis col 0,
        # act 4 sign-flipped. No gather copy needed.  poseR: (P,3,CJ,36)
        V.tensor_copy(out=poseR[:, :, :, 19:24:4], in_=poseR[:, :, :, 1:6:4])
        V.tensor_scalar_mul(poseR[:, :, :, 22], poseR[:, :, :, 22], -1.0)
        rot = poseR[:, :, :, 18:25]
        rots.append(rot)
        posV = poseR[:, :, :, 27:34]
        dl = delb[:, :, js]
        V.tensor_tensor(out=dl, in0=comR[:, :, js], in1=posV, op=OP.subtract)
        V.tensor_tensor(out=mw[:, :, js], in0=Mb, in1=dl, op=OP.mult)
        for k, (i, j) in enumerate([(0, 0), (1, 1), (2, 2), (0, 1), (0, 2), (1, 2)]):
            V.tensor_tensor(out=prod[:, k, js], in0=mw[:, i, js], in1=delb[:, j, js],
                            op=OP.mult)
        V.tensor_tensor(out=prod[:, 6:9, js], in0=Mb, in1=comR[:, :, js], op=OP.mult)
        for cx in range(3):
            y, z = (cx + 1) % 3, (cx + 2) % 3
            V.tensor_tensor(out=hphi[:, y, js], in0=rot[:, y], in1=delb[:, z, js],
                            op=OP.mult)
            V.tensor_tensor(out=hphi[:, z, js], in0=rot[:, z], in1=delb[:, y, js],
                            op=OP.mult)
            V.tensor_tensor(out=jac[:, cx, js], in0=hphi[:, y, js],
                            in1=hphi[:, z, js], op=OP.subtract)
        # act-reduction of the 9 product rows for this chunk
        V.tensor_reduce(out=red[:, :, js], in_=prod[:, :, js], axis=AX.X, op=OP.add)

    tu = prod[:, 0:3]
    tv = prod[:, 3:6]

    # c = scom/TM - BASE   (BASE is [0,0,bz])
    V.tensor_scalar(out=cv[:, 0:2], in0=red[:, 6:8], scalar1=1.0 / TM, scalar2=None,
                    op0=OP.mult)
    V.tensor_scalar(out=cv[:, 2], in0=red[:, 8], scalar1=1.0 / TM,
                    scalar2=float(BASE[2]), op0=OP.mult, op1=OP.subtract)

    # suffix cumsum over acts in place: mw becomes w
    for k in range(5, -1, -1):
        V.tensor_tensor(out=mw[:, :, :, k], in0=mw[:, :, :, k], in1=mw[:, :, :, k + 1],
                        op=OP.add)

    # w2 = w - SM∘c folds the former  Htheta = Hphi - c x jsm  stage into the
    # Hphi cross product:  Htheta = D⊙rot + (w - SM∘c) x jacob
    cvb3 = bc(cv[:].unsqueeze(3), (P, 3, J, N_ACT))
    V.tensor_tensor(out=tu[:], in0=SMb, in1=cvb3, op=OP.mult)
    V.tensor_tensor(out=mw[:], in0=mw[:], in1=tu[:], op=OP.subtract)

    # jsm = SM * jacob (reuses delb slot via tag)
    jsm = delb  # overwritten after last delb read (jacob products)
    V.tensor_tensor(out=jsm[:], in0=SMb, in1=jac[:], op=OP.mult)

    # Htheta = D*rot + w2 x jacob  (written into hphi)
    for cx in range(3):
        y, z = (cx + 1) % 3, (cx + 2) % 3
        V.tensor_tensor(out=tu[:, cx], in0=mw[:, y], in1=jac[:, z], op=OP.mult)
        V.tensor_tensor(out=tv[:, cx], in0=mw[:, z], in1=jac[:, y], op=OP.mult)
        V.tensor_tensor(out=hphi[:, cx], in0=tu[:, cx], in1=tv[:, cx], op=OP.subtract)
    DbC = bc(consts[:, 14:35].rearrange("p (c a) -> p c a", c=3).unsqueeze(2),
             (P, 3, CJ, N_ACT))
    for h in range(NCH):
        js = slice(h * CJ, (h + 1) * CJ)
        V.tensor_tensor(out=tu[:, :, js], in0=rots[h], in1=DbC, op=OP.mult)
        V.tensor_tensor(out=hphi[:, :, js], in0=hphi[:, :, js], in1=tu[:, :, js],
                        op=OP.add)

    # cc products and |c|^2, SS
    V.tensor_tensor(out=smalls[:, CC:CC + 3], in0=cv[:], in1=cv[:], op=OP.mult)
    for k, (i, j) in enumerate([(0, 1), (0, 2), (1, 2)]):
        V.tensor_tensor(out=smalls[:, CC + 3 + k], in0=cv[:, i], in1=cv[:, j],
                        op=OP.mult)
    V.tensor_reduce(out=smalls[:, SS_R], in_=red[:, 0:3].transpose([0, 2, 1]),
                    axis=AX.X, op=OP.add)
    V.tensor_reduce(out=smalls[:, CSQ_R], in_=smalls[:, CC:CC + 3].transpose([0, 2, 1]),
                    axis=AX.X, op=OP.add)

    csq_b = bc(smalls[:, CSQ_R].unsqueeze(1), (P, 3, J))
    ss_b = bc(smalls[:, SS_R].unsqueeze(1), (P, 3, J))

    # H_s diag rows HS..HS+2 ; off rows HS+3..HS+5
    a1 = smalls[:, M1_R:M1_R + 2]  # scratch pair rows (reused a lot below)
    V.tensor_tensor(out=smalls[:, T0_R:T0_R + 3], in0=smalls[:, CC:CC + 3], in1=csq_b,
                    op=OP.subtract)
    V.tensor_tensor(out=smalls[:, HS:HS + 3], in0=ss_b, in1=red[:, 0:3], op=OP.subtract)
    nc.vector.scalar_tensor_tensor(out=smalls[:, HS:HS + 3], in0=smalls[:, T0_R:T0_R + 3],
                                   scalar=TM, in1=smalls[:, HS:HS + 3],
                                   op0=OP.mult, op1=OP.add)
    V.tensor_tensor(out=smalls[:, HS:HS + 3], in0=smalls[:, HS:HS + 3], in1=CDb,
                    op=OP.add)
    nc.vector.scalar_tensor_tensor(out=smalls[:, HS + 3:HS + 6],
                                   in0=smalls[:, CC + 3:CC + 6], scalar=TM,
                                   in1=red[:, 3:6], op0=OP.mult, op1=OP.subtract)

    # adjugate (batched pairs via reversed/broadcast row views)
    h = lambda i: smalls[:, HS + i]
    hpair = lambda a, b: smalls[:, HS + a: (HS + b - 1 if b < a else HS + b + 1): (1 if b > a else -1)]
    b2 = lambda ap: bc(ap.unsqueeze(1), (P, 2, J))
    # a11 = h1 h2 - h5^2 ; a22 = h0 h2 - h4^2
    V.tensor_tensor(out=smalls[:, M1_R:M1_R + 2], in0=hpair(1, 0), in1=b2(h(2)), op=OP.mult)
    V.tensor_tensor(out=smalls[:, M2_R:M2_R + 2], in0=hpair(5, 4), in1=hpair(5, 4), op=OP.mult)
    V.tensor_tensor(out=smalls[:, ADJ:ADJ + 2], in0=smalls[:, M1_R:M1_R + 2],
                    in1=smalls[:, M2_R:M2_R + 2], op=OP.subtract)
    # a33 = h0 h1 - h3^2
    V.tensor_tensor(out=smalls[:, T0_R], in0=h(0), in1=h(1), op=OP.mult)
    V.tensor_tensor(out=smalls[:, T1_R], in0=h(3), in1=h(3), op=OP.mult)
    V.tensor_tensor(out=smalls[:, ADJ + 2], in0=smalls[:, T0_R], in1=smalls[:, T1_R],
                    op=OP.subtract)
    # a12 = h4 h5 - h3 h2 ; a13 = h3 h5 - h4 h1
    V.tensor_tensor(out=smalls[:, M1_R:M1_R + 2], in0=hpair(4, 3), in1=b2(h(5)), op=OP.mult)
    V.tensor_tensor(out=smalls[:, M2_R:M2_R + 2], in0=hpair(3, 4), in1=hpair(2, 1), op=OP.mult)
    V.tensor_tensor(out=smalls[:, ADJ + 3:ADJ + 5], in0=smalls[:, M1_R:M1_R + 2],
                    in1=smalls[:, M2_R:M2_R + 2], op=OP.subtract)
    # a23 = h3 h4 - h0 h5
    V.tensor_tensor(out=smalls[:, T0_R], in0=h(3), in1=h(4), op=OP.mult)
    V.tensor_tensor(out=smalls[:, T1_R], in0=h(0), in1=h(5), op=OP.mult)
    V.tensor_tensor(out=smalls[:, ADJ + 5], in0=smalls[:, T0_R], in1=smalls[:, T1_R],
                    op=OP.subtract)

    # det = h0 a11 + h3 a12 + h4 a13 ; A = adj * (-1/det)
    V.tensor_tensor(out=smalls[:, T0_R], in0=h(0), in1=smalls[:, ADJ], op=OP.mult)
    V.tensor_tensor(out=smalls[:, T1_R], in0=h(3), in1=smalls[:, ADJ + 3], op=OP.mult)
    V.tensor_tensor(out=smalls[:, T2_R], in0=h(4), in1=smalls[:, ADJ + 4], op=OP.mult)
    V.tensor_tensor(out=smalls[:, DET_R], in0=smalls[:, T0_R], in1=smalls[:, T1_R],
                    op=OP.add)
    V.tensor_tensor(out=smalls[:, DET_R], in0=smalls[:, DET_R], in1=smalls[:, T2_R],
                    op=OP.add)
    V.reciprocal(out=smalls[:, RDN_R], in_=smalls[:, DET_R])
    rdn_b = bc(smalls[:, RDN_R].unsqueeze(1), (P, 6, J))
    nc.vector.scalar_tensor_tensor(out=smalls[:, ADJ:ADJ + 6],
                                   in0=smalls[:, ADJ:ADJ + 6], scalar=-1.0,
                                   in1=rdn_b, op0=OP.mult, op1=OP.mult)

    # bot = A @ Htheta -> outb cols 21..41 ; top = -(1/TM) jsm + c x bot ->
    # cols 0..20.  Done in two j-halves so the first half's output DMA
    # (168B-contiguous runs) overlaps the second half's compute.
    Arows = [[0, 3, 4], [3, 1, 5], [4, 5, 2]]
    JH = J // 2
    for g in range(2):
        gs = slice(g * JH, (g + 1) * JH)
        cvb = lambda i: bc(cv[:, i, gs].unsqueeze(2), (P, JH, N_ACT))
        Ab = lambda r: bc(smalls[:, ADJ + r, gs].unsqueeze(2), (P, JH, N_ACT))
        bot = lambda c: outb[:, gs, 21 + 7 * c: 28 + 7 * c]
        for oc in range(3):
            r0, r1, r2 = Arows[oc]
            V.tensor_tensor(out=tu[:, 0, gs], in0=Ab(r0), in1=hphi[:, 0, gs], op=OP.mult)
            V.tensor_tensor(out=tu[:, 1, gs], in0=Ab(r1), in1=hphi[:, 1, gs], op=OP.mult)
            V.tensor_tensor(out=tu[:, 2, gs], in0=tu[:, 0, gs], in1=tu[:, 1, gs], op=OP.add)
            V.tensor_tensor(out=tu[:, 0, gs], in0=Ab(r2), in1=hphi[:, 2, gs], op=OP.mult)
            V.tensor_tensor(out=bot(oc), in0=tu[:, 2, gs], in1=tu[:, 0, gs], op=OP.add)
        for cx in range(3):
            y, z = (cx + 1) % 3, (cx + 2) % 3
            V.tensor_tensor(out=tu[:, cx, gs], in0=cvb(y), in1=bot(z), op=OP.mult)
            nc.vector.scalar_tensor_tensor(out=tv[:, cx, gs], in0=jsm[:, cx, gs],
                                           scalar=-1.0 / TM, in1=tu[:, cx, gs],
                                           op0=OP.mult, op1=OP.add)
            V.tensor_tensor(out=tu[:, cx, gs], in0=cvb(z), in1=bot(y), op=OP.mult)
            V.tensor_tensor(out=outb[:, gs, 7 * cx: 7 * cx + 7], in0=tv[:, cx, gs],
                            in1=tu[:, cx, gs], op=OP.subtract)
        nc.sync.dma_start(out=dram["out"][:, gs], in_=outb[:, gs])


@functools.lru_cache(maxsize=1)
def _program():
    from contextlib import ExitStack
    import concourse.bacc as bacc
    import concourse.tile as tile
    from concourse import mybir

    f32 = mybir.dt.float32
    nc = bacc.Bacc("TRN2", target_bir_lowering=False, debug=False)
    dram = {
        "com": nc.dram_tensor("com", [P, J, 21], f32, kind="ExternalInput"),
        "pose": nc.dram_tensor("pose", [P, J, 144], f32, kind="ExternalInput"),
        "consts": nc.dram_tensor("consts", [P, NCONST], f32, kind="ExternalInput"),
        "out": nc.dram_tensor("out", [P, J, 42], f32, kind="ExternalOutput"),
    }
    with tile.TileContext(nc) as tc:
        with ExitStack() as ctx:
            _emit(nc, tc, ctx, dram)
    nc.compile()
    return nc


def _kernel_bm0(com, pose):
    # bm=0 path (not exercised by the shipped setup_inputs; numpy fallback)
    rot = pose[:, :, :3, 2, :N_ACT].copy()
    rot[..., 1] = pose[:, :, :3, 0, 1]
    rot[..., 5] = pose[:, :, :3, 0, 5]
    rot[..., 4] *= -1.0
    delp = pose[:, :, :3, 3, -2][..., None] - pose[:, :, :3, 3, :N_ACT]
    jt = np.cross(rot, delp, axis=2)
    return np.concatenate([jt, rot], axis=2).astype(np.float32)


def kernel(com_list, link_pose_list, bm):
    com_list = np.ascontiguousarray(com_list, dtype=np.float32)
    link_pose_list = np.ascontiguousarray(link_pose_list, dtype=np.float32)
    if not int(bm):
        return _kernel_bm0(com_list, link_pose_list)

    from concourse.bass_utils import run_bass_kernel_spmd

    nc = _program()
    com_flat = com_list.reshape(N_CORES, P, J, 21)
    pose_flat = link_pose_list.reshape(N_CORES, P, J, 144)
    consts = np.broadcast_to(CONSTS, (P, NCONST)).copy()
    in_maps = [
        {"com": com_flat[k], "pose": pose_flat[k], "consts": consts}
        for k in range(N_CORES)
    ]
    res = run_bass_kernel_spmd(nc, in_maps, core_ids=list(range(N_CORES)))
    out = np.stack([res.results[k]["out"] for k in range(N_CORES)])
    return out.reshape(512, 256, 6, 7)



# revision 2
# speedup vs baseline: 1.7382x; 1.7382x over previous
"""Trainium2 Bass kernel for nn_CanadarmJacob (space-arm Jacobian, bm=1 path).

Contract: kernel(**inputs) takes FULL inputs (com_list (512,256,3,7) f32,
link_pose_list (512,256,4,4,9) f32, bm scalar) and returns the FULL output
(512,256,6,7) f32. Internally shards samples across 8 NeuronCores (pure data
parallel), runs a Bass/Tile kernel per core, and gathers.

Design (v2): fp16 datapath (DVE 2x_1p mode on tensor_tensor), plane layout
(P=128 partitions x comp x J=128 samples, J innermost/contiguous). Mass and
inertia constants pre-scaled by SC=1/64 so intermediates stay inside fp16
range; the scale cancels in bot = A @ Htheta because A inverts the scaled
H_s. The small per-sample 3x3-inverse chain runs in fp32 (J-length rows).

Math per sample (act dim a=0..6, coords c in {x,y,z}):
  del   = com - pos
  mdel  = (M*SC) . del            jac = rot x del
  S6    = sum_a mdel_i del_j      scom = sum_a (M*SC) com_a
  c     = scom/(TM*SC) - BASE
  w     = suffix-cumsum(mdel);    w2 = w - (SM*SC) c
  Hth   = (D_suf*SC).rot + w2 x jac
  H_s   = TMS(cc^T - |c|^2 I) + CD*SC + (trS)I - S   (scaled)
  A     = -(H_s)^(-1)             bot = A @ Hth   (scales cancel)
  top   = -(SM/TM).jac + c x bot
  out   = [top; bot]
"""
import sys
import functools

if "/opt/trn_rl_repo" not in sys.path:
    sys.path.insert(0, "/opt/trn_rl_repo")

import numpy as np

# ---------------------------------------------------------------- constants
N_CORES = 8
P = 128          # SBUF partitions
J = 128          # samples per partition per core
N_ACT = 7
SC = 1.0 / 64.0  # range scale for fp16

MASS = np.array([105.98, 105.98, 314.98, 279.2, 105.98, 105.98, 243.66], np.float64)
TM = float(MASS.sum() + 100000.0 + 243.66)
DIAGS = np.array([[12.19, 12.19, 3.061], [12.19, 12.19, 3.061], [15.41, 2094.71, 2103.19],
                  [9.522, 1966.28, 1966.28], [8.305, 3.061, 8.0386], [12.13, 12.13, 3.061],
                  [9.336, 44.41, 44.41]], np.float64)
D_SUF = np.cumsum(DIAGS[::-1], axis=0)[::-1]          # (7,3) suffix inertia diag
SM = np.cumsum(MASS[::-1])[::-1]                      # (7,) suffix mass
CD = DIAGS.sum(axis=0)                                # (3,)
_TF0 = np.array([[1, 0, 0, 0], [0, -1, 0, 0], [0, 0, 1.3, 6], [0, 0, 0, 1]], np.float64)
_COM0 = np.array([[1, 0, 0, 0], [0, 1, 0, 0], [0, 0, 1, 0.5], [0, 0, 0, 1]], np.float64)
BASE = (_TF0 @ _COM0)[:3, 3] * 243.66 / (100000.0 + 243.66)   # [0, 0, ~0.0162]

TMS = TM * SC
CDS = CD * SC
AXIS = np.array([2, 0, 2, 2, 2, 0, 2])
LINK = np.arange(N_ACT)
SIGN = np.array([1., 1., 1., 1., -1., 1., 1.], np.float32)


def _emit(nc, tc, ctx, dram):
    from concourse import mybir

    f16 = mybir.dt.float16
    f32 = mybir.dt.float32
    OP = mybir.AluOpType
    V = nc.vector

    pool = ctx.enter_context(tc.tile_pool(name="main", bufs=1))

    # fp16 tiles; *_E tiles carry c-plane wrap-around extension [x,y,z,x,y]
    consts = pool.tile([P, 3, 7, J], f16)   # rows: M*SC | SM*SC | -SM/TM
    rot = pool.tile([P, 5, 7, J], f16)
    drot = pool.tile([P, 3, 7, J], f16)
    pos = pool.tile([P, 3, 7, J], f16)
    com = pool.tile([P, 3, 7, J], f16)
    delE = pool.tile([P, 5, 7, J], f16)
    mdel = pool.tile([P, 3, 7, J], f16)     # becomes w after suffix cumsum
    scr1 = pool.tile([P, 3, 7, J], f16)
    scr2 = pool.tile([P, 3, 7, J], f16)
    jacE = pool.tile([P, 5, 7, J], f16)
    prod = pool.tile([P, 9, 7, J], f16)     # Sdiag | Soff | mcom products
    tscr = pool.tile([P, 9, 3, J], f16)
    red16 = pool.tile([P, 9, J], f16)
    vscr = pool.tile([P, 9, J], f16)
    smc = pool.tile([P, 3, 7, J], f16)
    w2E = pool.tile([P, 5, 7, J], f16)
    jsm = pool.tile([P, 3, 7, J], f16)
    hthE = pool.tile([P, 5, 7, J], f16)
    outE = pool.tile([P, 8, 7, J], f16)     # top(0:3) | bot(3:6) | bot-ext(6:8)
    c16 = pool.tile([P, 5, J], f16)
    A16 = pool.tile([P, 8, J], f16)         # a11,a22,a33,a12,a23,a13,a12,a23
    # fp32 smalls
    red32 = pool.tile([P, 9, J], f32)       # Sd(xx,yy,zz) | So(xy,yz,zx) | scom
    c32 = pool.tile([P, 5, J], f32)
    sm32 = pool.tile([P, 8, J], f32)
    hs = pool.tile([P, 6, J], f32)          # hxx,hyy,hzz,hxy,hyz,hzx
    adj = pool.tile([P, 6, J], f32)         # a11,a22,a33,a12,a23,a13
    A32 = pool.tile([P, 6, J], f32)

    nc.sync.dma_start(out=pos[:], in_=dram["pos"][:])
    nc.sync.dma_start(out=com[:], in_=dram["com"][:])
    nc.sync.dma_start(out=consts[:], in_=dram["consts"][:])
    nc.sync.dma_start(out=rot[:], in_=dram["rot"][:])
    nc.sync.dma_start(out=drot[:], in_=dram["drot"][:])

    def bc_c(ap):   # (P,7,J) -> (P,3,7,J), broadcast over coord planes
        return ap.unsqueeze(1).broadcast_to((P, 3, 7, J))

    def bc_a(ap):   # (P,3,J) -> (P,3,7,J), broadcast over act
        return ap.unsqueeze(2).broadcast_to((P, 3, 7, J))

    Mb = bc_c(consts[:, 0])
    SMb = bc_c(consts[:, 1])
    SMTb = bc_c(consts[:, 2])

    # --- del, mdel, jac --------------------------------------------------
    V.tensor_tensor(out=delE[:, 0:3], in0=com[:], in1=pos[:], op=OP.subtract)
    V.tensor_copy(out=delE[:, 3:5], in_=delE[:, 0:2])
    V.tensor_tensor(out=mdel[:], in0=Mb, in1=delE[:, 0:3], op=OP.mult)
    V.tensor_tensor(out=scr1[:], in0=rot[:, 1:4], in1=delE[:, 2:5], op=OP.mult)
    V.tensor_tensor(out=scr2[:], in0=rot[:, 2:5], in1=delE[:, 1:4], op=OP.mult)
    V.tensor_tensor(out=jacE[:, 0:3], in0=scr1[:], in1=scr2[:], op=OP.subtract)
    V.tensor_copy(out=jacE[:, 3:5], in_=jacE[:, 0:2])

    # --- S products + mcom, act-reduction tree --------------------------
    V.tensor_tensor(out=prod[:, 0:3], in0=mdel[:], in1=delE[:, 0:3], op=OP.mult)
    V.tensor_tensor(out=prod[:, 3:6], in0=mdel[:], in1=delE[:, 1:4], op=OP.mult)
    V.tensor_tensor(out=prod[:, 6:9], in0=Mb, in1=com[:], op=OP.mult)
    V.tensor_tensor(out=tscr[:], in0=prod[:, :, 0:3], in1=prod[:, :, 3:6], op=OP.add)
    V.tensor_tensor(out=red16[:], in0=tscr[:, :, 0], in1=tscr[:, :, 1], op=OP.add)
    V.tensor_tensor(out=vscr[:], in0=tscr[:, :, 2], in1=prod[:, :, 6], op=OP.add)
    V.tensor_tensor(out=red16[:], in0=red16[:], in1=vscr[:], op=OP.add)
    V.tensor_copy(out=red32[:], in_=red16[:])

    # --- fp32 smalls: c, H_s, adjugate, A --------------------------------
    inv_tms = 1.0 / TMS
    V.tensor_scalar(out=c32[:, 0:2], in0=red32[:, 6:8], scalar1=inv_tms,
                    scalar2=None, op0=OP.mult)
    V.tensor_scalar(out=c32[:, 2], in0=red32[:, 8], scalar1=inv_tms,
                    scalar2=float(BASE[2]), op0=OP.mult, op1=OP.subtract)
    V.tensor_copy(out=c32[:, 3:5], in_=c32[:, 0:2])
    V.tensor_copy(out=c16[:], in_=c32[:])

    # sm32 rows: 0=SS 1=q 2=csq 3..7 scratch
    V.tensor_tensor(out=sm32[:, 0], in0=red32[:, 0], in1=red32[:, 1], op=OP.add)
    V.tensor_tensor(out=sm32[:, 0], in0=sm32[:, 0], in1=red32[:, 2], op=OP.add)
    ccd = sm32[:, 5:8]
    V.tensor_tensor(out=ccd, in0=c32[:, 0:3], in1=c32[:, 0:3], op=OP.mult)
    V.tensor_tensor(out=sm32[:, 2], in0=sm32[:, 5], in1=sm32[:, 6], op=OP.add)
    V.tensor_tensor(out=sm32[:, 2], in0=sm32[:, 2], in1=sm32[:, 7], op=OP.add)
    # q = SS - TMS*csq
    V.scalar_tensor_tensor(out=sm32[:, 1], in0=sm32[:, 2], scalar=-TMS,
                           in1=sm32[:, 0], op0=OP.mult, op1=OP.add)
    # hs diag: TMS*cc_d + CD_c, then + (q - Sd)
    for cx in range(3):
        V.tensor_scalar(out=hs[:, cx], in0=sm32[:, 5 + cx], scalar1=TMS,
                        scalar2=float(CDS[cx]), op0=OP.mult, op1=OP.add)
    qb = sm32[:, 1].unsqueeze(1).broadcast_to((P, 3, J))
    V.tensor_tensor(out=sm32[:, 3:6], in0=qb, in1=red32[:, 0:3], op=OP.subtract)
    V.tensor_tensor(out=hs[:, 0:3], in0=hs[:, 0:3], in1=sm32[:, 3:6], op=OP.add)
    # hs off: TMS*cc_o - So  (cc_o = c * c[[y,z,x]])
    V.tensor_tensor(out=sm32[:, 3:6], in0=c32[:, 0:3], in1=c32[:, 1:4], op=OP.mult)
    V.tensor_scalar(out=sm32[:, 3:6], in0=sm32[:, 3:6], scalar1=TMS,
                    scalar2=None, op0=OP.mult)
    V.tensor_tensor(out=hs[:, 3:6], in0=sm32[:, 3:6], in1=red32[:, 3:6], op=OP.subtract)

    # adjugate: adj rows [a11,a22,a33,a12,a23,a13]
    h2b = hs[:, 2].unsqueeze(1).broadcast_to((P, 2, J))
    h3b = hs[:, 3].unsqueeze(1).broadcast_to((P, 2, J))
    # (a11,a22) = [h1,h0]*h2 - [h4,h5]^2
    V.tensor_tensor(out=sm32[:, 3:5], in0=hs[:, 1::-1], in1=h2b, op=OP.mult)
    V.tensor_tensor(out=sm32[:, 5:7], in0=hs[:, 4:6], in1=hs[:, 4:6], op=OP.mult)
    V.tensor_tensor(out=adj[:, 0:2], in0=sm32[:, 3:5], in1=sm32[:, 5:7], op=OP.subtract)
    # a33 = h0h1 - h3^2
    V.tensor_tensor(out=sm32[:, 3], in0=hs[:, 0], in1=hs[:, 1], op=OP.mult)
    V.tensor_tensor(out=sm32[:, 4], in0=hs[:, 3], in1=hs[:, 3], op=OP.mult)
    V.tensor_tensor(out=adj[:, 2], in0=sm32[:, 3], in1=sm32[:, 4], op=OP.subtract)
    # (a23,a13) = h3*[h5,h4] - [h0,h1]*[h4,h5]
    V.tensor_tensor(out=sm32[:, 3:5], in0=h3b, in1=hs[:, 5:3:-1], op=OP.mult)
    V.tensor_tensor(out=sm32[:, 5:7], in0=hs[:, 0:2], in1=hs[:, 4:6], op=OP.mult)
    V.tensor_tensor(out=adj[:, 4:6], in0=sm32[:, 3:5], in1=sm32[:, 5:7], op=OP.subtract)
    # a12 = h4h5 - h3h2
    V.tensor_tensor(out=sm32[:, 3], in0=hs[:, 4], in1=hs[:, 5], op=OP.mult)
    V.tensor_tensor(out=sm32[:, 4], in0=hs[:, 3], in1=hs[:, 2], op=OP.mult)
    V.tensor_tensor(out=adj[:, 3], in0=sm32[:, 3], in1=sm32[:, 4], op=OP.subtract)
    # det = h0*a11 + h3*a12 + h5*a13
    V.tensor_tensor(out=sm32[:, 3], in0=hs[:, 0], in1=adj[:, 0], op=OP.mult)
    V.tensor_tensor(out=sm32[:, 4], in0=hs[:, 3], in1=adj[:, 3], op=OP.mult)
    V.tensor_tensor(out=sm32[:, 5], in0=hs[:, 5], in1=adj[:, 5], op=OP.mult)
    V.tensor_tensor(out=sm32[:, 6], in0=sm32[:, 3], in1=sm32[:, 4], op=OP.add)
    V.tensor_tensor(out=sm32[:, 6], in0=sm32[:, 6], in1=sm32[:, 5], op=OP.add)
    V.reciprocal(out=sm32[:, 7], in_=sm32[:, 6])
    V.tensor_scalar(out=sm32[:, 7], in0=sm32[:, 7], scalar1=-1.0, scalar2=None,
                    op0=OP.mult)
    rdb = sm32[:, 7].unsqueeze(1).broadcast_to((P, 6, J))
    V.tensor_tensor(out=A32[:], in0=adj[:], in1=rdb, op=OP.mult)
    V.tensor_copy(out=A16[:, 0:6], in_=A32[:])
    V.tensor_copy(out=A16[:, 6:8], in_=A16[:, 3:5])

    # --- w = suffix cumsum(mdel); w2 = w - SM*c --------------------------
    for k in range(5, -1, -1):
        V.tensor_tensor(out=mdel[:, :, k], in0=mdel[:, :, k], in1=mdel[:, :, k + 1],
                        op=OP.add)
    V.tensor_tensor(out=smc[:], in0=SMb, in1=bc_a(c16[:, 0:3]), op=OP.mult)
    V.tensor_tensor(out=w2E[:, 0:3], in0=mdel[:], in1=smc[:], op=OP.subtract)
    V.tensor_copy(out=w2E[:, 3:5], in_=w2E[:, 0:2])

    # --- jsm, Htheta -----------------------------------------------------
    V.tensor_tensor(out=jsm[:], in0=SMTb, in1=jacE[:, 0:3], op=OP.mult)
    V.tensor_tensor(out=scr1[:], in0=w2E[:, 1:4], in1=jacE[:, 2:5], op=OP.mult)
    V.tensor_tensor(out=scr2[:], in0=w2E[:, 2:5], in1=jacE[:, 1:4], op=OP.mult)
    V.tensor_tensor(out=hthE[:, 0:3], in0=scr1[:], in1=scr2[:], op=OP.subtract)
    V.tensor_tensor(out=hthE[:, 0:3], in0=hthE[:, 0:3], in1=drot[:], op=OP.add)
    V.tensor_copy(out=hthE[:, 3:5], in_=hthE[:, 0:2])

    # --- bot = A @ Hth ---------------------------------------------------
    def Ab(r):
        return A16[:, r:r + 3].unsqueeze(2).broadcast_to((P, 3, 7, J))

    V.tensor_tensor(out=scr1[:], in0=Ab(0), in1=hthE[:, 0:3], op=OP.mult)
    V.tensor_tensor(out=scr2[:], in0=Ab(3), in1=hthE[:, 1:4], op=OP.mult)
    V.tensor_tensor(out=scr1[:], in0=scr1[:], in1=scr2[:], op=OP.add)
    V.tensor_tensor(out=scr2[:], in0=Ab(5), in1=hthE[:, 2:5], op=OP.mult)
    V.tensor_tensor(out=outE[:, 3:6], in0=scr1[:], in1=scr2[:], op=OP.add)
    V.tensor_copy(out=outE[:, 6:8], in_=outE[:, 3:5])
    nc.sync.dma_start(out=dram["out"][:, 3:6], in_=outE[:, 3:6])

    # --- top = c x bot + jsm --------------------------------------------
    V.tensor_tensor(out=scr1[:], in0=bc_a(c16[:, 1:4]), in1=outE[:, 5:8], op=OP.mult)
    V.tensor_tensor(out=scr2[:], in0=bc_a(c16[:, 2:5]), in1=outE[:, 4:7], op=OP.mult)
    V.tensor_tensor(out=scr1[:], in0=scr1[:], in1=scr2[:], op=OP.subtract)
    V.tensor_tensor(out=outE[:, 0:3], in0=scr1[:], in1=jsm[:], op=OP.add)
    nc.sync.dma_start(out=dram["out"][:, 0:3], in_=outE[:, 0:3])


@functools.lru_cache(maxsize=1)
def _program():
    from contextlib import ExitStack
    import concourse.bacc as bacc
    import concourse.tile as tile
    from concourse import mybir

    f16 = mybir.dt.float16
    nc = bacc.Bacc("TRN2", target_bir_lowering=False, debug=False)
    dram = {
        "rot": nc.dram_tensor("rot", [P, 5, 7, J], f16, kind="ExternalInput"),
        "drot": nc.dram_tensor("drot", [P, 3, 7, J], f16, kind="ExternalInput"),
        "pos": nc.dram_tensor("pos", [P, 3, 7, J], f16, kind="ExternalInput"),
        "com": nc.dram_tensor("com", [P, 3, 7, J], f16, kind="ExternalInput"),
        "consts": nc.dram_tensor("consts", [P, 3, 7, J], f16, kind="ExternalInput"),
        "out": nc.dram_tensor("out", [P, 6, 7, J], f16, kind="ExternalOutput"),
    }
    with tile.TileContext(nc) as tc:
        with ExitStack() as ctx:
            _emit(nc, tc, ctx, dram)
    nc.compile()
    return nc


def _prep(com_list, link_pose_list):
    """Host-side packing: gather rot, scale drot, fp16-cast, plane layout."""
    n = N_CORES * P * J
    com = np.ascontiguousarray(com_list, dtype=np.float32).reshape(n, 3, 7)
    pose = np.ascontiguousarray(link_pose_list, dtype=np.float32).reshape(n, 4, 4, 9)
    rot = pose[:, :3, AXIS, LINK] * SIGN                 # (n,3,7)
    rot5 = np.concatenate([rot, rot[:, 0:2]], axis=1)    # (n,5,7)
    drot = rot * (D_SUF.T * SC).astype(np.float32)
    pos = pose[:, :3, 3, :N_ACT]

    def pk(x, r):
        x16 = x.astype(np.float16).reshape(N_CORES, P, J, r, 7)
        return np.ascontiguousarray(x16.transpose(0, 1, 3, 4, 2))

    rotp, drotp, posp, comp = pk(rot5, 5), pk(drot, 3), pk(pos, 3), pk(com, 3)
    constsv = np.concatenate([MASS * SC, SM * SC, -SM / TM]).astype(np.float16)
    constsp = np.ascontiguousarray(
        np.broadcast_to(constsv.reshape(1, 3, 7, 1), (P, 3, 7, J)))
    return [{"rot": rotp[k], "drot": drotp[k], "pos": posp[k], "com": comp[k],
             "consts": constsp} for k in range(N_CORES)]


def _post(res):
    out = np.stack([res.results[k]["out"] for k in range(N_CORES)])  # (8,P,6,7,J)
    out = out.astype(np.float32).transpose(0, 1, 4, 2, 3)            # (8,P,J,6,7)
    return np.ascontiguousarray(out.reshape(512, 256, 6, 7))


def _kernel_bm0(com, pose):
    # bm=0 path (not exercised by the shipped setup_inputs; numpy fallback)
    rot = pose[:, :, :3, 2, :N_ACT].copy()
    rot[..., 1] = pose[:, :, :3, 0, 1]
    rot[..., 5] = pose[:, :, :3, 0, 5]
    rot[..., 4] *= -1.0
    delp = pose[:, :, :3, 3, -2][..., None] - pose[:, :, :3, 3, :N_ACT]
    jt = np.cross(rot, delp, axis=2)
    return np.concatenate([jt, rot], axis=2).astype(np.float32)


def kernel(com_list, link_pose_list, bm):
    com_list = np.ascontiguousarray(com_list, dtype=np.float32)
    link_pose_list = np.ascontiguousarray(link_pose_list, dtype=np.float32)
    if not int(bm):
        return _kernel_bm0(com_list, link_pose_list)

    from concourse.bass_utils import run_bass_kernel_spmd

    nc = _program()
    in_maps = _prep(com_list, link_pose_list)
    res = run_bass_kernel_spmd(nc, in_maps, core_ids=list(range(N_CORES)))
    return _post(res)
